# revision 1
# baseline (speedup 1.0000x reference)
"""Trainium2 Bass kernel for nn_CharDecoder.

Teacher-forced character LSTM decoder:
  h0 = qr @ Wp.T + bp; c0 = 0
  per step: gates = x @ W_ih.T + h @ W_hh.T + b; LSTM cell; logits = h @ Wo.T + bo

Strategy:
 - Data-parallel over the flattened B*W = 4096 rows: 512 rows per core x 8 cores.
 - Everything on-device lives in transposed ("feature-major") layout
   [feature, row]: gatesT = W_ih @ xT + W_hh @ hT. The LSTM nonlinearities
   then produce hT directly in the lhs-contraction layout the next step's
   matmul needs -> no per-step transposes. The per-gate bias is a
   per-partition constant in this layout, folded into the ScalarE
   activation for free.
 - Matmuls run in float32r (full fp32 storage, reduced-precision PE mode):
   measured ~269 ns per [128x128]x[128x512] matmul vs ~1016 ns for fp32,
   with ~1e-4 relative error (vs ~2e-3 for bf16).
 - One-hot inputs are built on the host (same construction as the
   reference's jax.nn.one_hot input encoding); the matmuls consuming them
   run on device.
"""

import numpy as np
import jax
from jax.sharding import Mesh, PartitionSpec
from jax.experimental.shard_map import shard_map

import concourse.bacc as bacc
import concourse.mybir as mybir
import concourse.tile as tile
import concourse.bass2jax as bass2jax

B, W, Q, H, A, C = 32, 128, 256, 512, 128, 16
NCORES = 8
R = B * W // NCORES          # 512 rows per core
KH = H // 128                # 4 contraction chunks over H
KQ = Q // 128                # 2 contraction chunks over Q
NG = 4 * H // 128            # 16 gate-dim chunks (i,f,g,o x 4)

F32 = mybir.dt.float32
F32R = mybir.dt.float32r
SIG = mybir.ActivationFunctionType.Sigmoid
TANH = mybir.ActivationFunctionType.Tanh
IDENT = mybir.ActivationFunctionType.Identity


def build_nc(repeat=1):
    nc = bacc.Bacc("TRN2", target_bir_lowering=False, debug=False, num_devices=NCORES)

    qrT_d = nc.dram_tensor("qrT", [KQ, 128, R], F32R, kind="ExternalInput").ap()
    xT_d = nc.dram_tensor("xT", [C, 128, R], F32R, kind="ExternalInput").ap()
    wpT_d = nc.dram_tensor("wpT", [KQ, 128, H], F32R, kind="ExternalInput").ap()
    wihT_d = nc.dram_tensor("wihT", [128, 4 * H], F32R, kind="ExternalInput").ap()
    whhT_d = nc.dram_tensor("whhT", [KH, 128, 4 * H], F32R, kind="ExternalInput").ap()
    woT_d = nc.dram_tensor("woT", [KH, 128, A], F32R, kind="ExternalInput").ap()
    bg_d = nc.dram_tensor("bg", [128, NG], F32, kind="ExternalInput").ap()
    bp_d = nc.dram_tensor("bp", [128, KH], F32, kind="ExternalInput").ap()
    bo_d = nc.dram_tensor("bo", [128, 1], F32, kind="ExternalInput").ap()
    out_d = nc.dram_tensor("outT", [C, 128, R], F32, kind="ExternalOutput").ap()

    with tile.TileContext(nc) as tc:
        with (
            tc.tile_pool(name="wpool", bufs=1) as wp,
            tc.tile_pool(name="hcpool", bufs=2) as hp,
            tc.tile_pool(name="gpool", bufs=1) as gp,
            tc.tile_pool(name="lpool", bufs=2) as lp,
            tc.tile_pool(name="pgates", bufs=6, space="PSUM") as pg,
            tc.tile_pool(name="plog", bufs=2, space="PSUM") as pl,
        ):
            # ---- persistent loads ----
            whh = []
            for k in range(KH):
                t_ = wp.tile([128, 4 * H], F32R, name=f"whh_{k}", tag=f"whh{k}")
                nc.sync.dma_start(out=t_, in_=whhT_d[k])
                whh.append(t_)
            wih = wp.tile([128, 4 * H], F32R, name="wih", tag="wih")
            nc.sync.dma_start(out=wih, in_=wihT_d)
            wpt = []
            for q in range(KQ):
                t_ = wp.tile([128, H], F32R, name=f"wpt_{q}", tag=f"wpt{q}")
                nc.sync.dma_start(out=t_, in_=wpT_d[q])
                wpt.append(t_)
            wo = []
            for k in range(KH):
                t_ = wp.tile([128, A], F32R, name=f"wo_{k}", tag=f"wo{k}")
                nc.sync.dma_start(out=t_, in_=woT_d[k])
                wo.append(t_)
            qr = []
            for q in range(KQ):
                t_ = wp.tile([128, R], F32R, name=f"qr_{q}", tag=f"qr{q}")
                nc.sync.dma_start(out=t_, in_=qrT_d[q])
                qr.append(t_)
            xts = []
            for t in range(C):
                t_ = wp.tile([128, R], F32R, name=f"xt_{t}", tag=f"xt{t}")
                nc.sync.dma_start(out=t_, in_=xT_d[t])
                xts.append(t_)
            bg = wp.tile([128, NG], F32, name="bg", tag="bg")
            nc.sync.dma_start(out=bg, in_=bg_d)
            bp = wp.tile([128, KH], F32, name="bp", tag="bp")
            nc.sync.dma_start(out=bp, in_=bp_d)
            bo = wp.tile([128, 1], F32, name="bo", tag="bo")
            nc.sync.dma_start(out=bo, in_=bo_d)

            for rep in range(repeat):
                # ---- h0 = Wp @ qrT + bp ; c0 = 0 ----
                h = [None] * KH
                c = [None] * KH
                for j in range(KH):
                    ph = pg.tile([128, R], F32, name=f"ph0_{j}_{rep}", tag="gp")
                    for q in range(KQ):
                        nc.tensor.matmul(
                            ph, wpt[q][:, j * 128:(j + 1) * 128], qr[q],
                            start=(q == 0), stop=(q == KQ - 1),
                        )
                    hj = hp.tile([128, R], F32R, name=f"h0_{j}_{rep}", tag=f"h{j}")
                    nc.scalar.activation(hj, ph, IDENT, bias=bp[:, j:j + 1])
                    h[j] = hj
                    cj = hp.tile([128, R], F32, name=f"c0_{j}_{rep}", tag=f"c{j}")
                    nc.vector.memset(cj, 0.0)
                    c[j] = cj

                prev_logits = None  # (psum_tile, step) pending bias-add + store

                def emit_gate_group(m, xt, hin, name):
                    ps = pg.tile([128, R], F32, name=name, tag="gp")
                    nc.tensor.matmul(
                        ps, wih[:, m * 128:(m + 1) * 128], xt, start=True, stop=False
                    )
                    for k in range(KH):
                        nc.tensor.matmul(
                            ps, whh[k][:, m * 128:(m + 1) * 128], hin[k],
                            start=False, stop=(k == KH - 1),
                        )
                    return ps

                def flush_logits():
                    nonlocal prev_logits
                    if prev_logits is None:
                        return
                    ps_l, t_l = prev_logits
                    lsb = lp.tile([128, R], F32, name=f"lsb_{t_l}_{rep}", tag="lsb")
                    nc.scalar.activation(lsb, ps_l, IDENT, bias=bo[:, 0:1])
                    nc.sync.dma_start(out=out_d[t_l], in_=lsb)
                    prev_logits = None

                for t in range(C):
                    xt = xts[t]
                    hn = [None] * KH
                    cn = [None] * KH
                    tanh_c = [None] * KH
                    for j in range(KH):
                        # i, f, g chunks for this H-slice j
                        ps_i = emit_gate_group(0 * KH + j, xt, h, f"pi_{t}_{j}_{rep}")
                        if t > 0 and j == 0:
                            # previous step's logits matmuls slot in here: by
                            # now all h chunks of step t-1 are ready, so the
                            # PE never stalls on them.
                            ps_l = pl.tile([128, R], F32, name=f"plog_{t-1}_{rep}", tag="pl")
                            for k in range(KH):
                                nc.tensor.matmul(
                                    ps_l, wo[k], h[k], start=(k == 0), stop=(k == KH - 1)
                                )
                            flush_logits()
                            prev_logits = (ps_l, t - 1)
                        si = gp.tile([128, R], F32, name=f"si_{t}_{j}_{rep}", tag=f"si{j}")
                        nc.scalar.activation(si, ps_i, SIG, bias=bg[:, 0 * KH + j:0 * KH + j + 1])
                        ps_f = emit_gate_group(1 * KH + j, xt, h, f"pf_{t}_{j}_{rep}")
                        sf = gp.tile([128, R], F32, name=f"sf_{t}_{j}_{rep}", tag=f"sf{j}")
                        nc.scalar.activation(sf, ps_f, SIG, bias=bg[:, 1 * KH + j:1 * KH + j + 1])
                        ps_g = emit_gate_group(2 * KH + j, xt, h, f"pg_{t}_{j}_{rep}")
                        tg = gp.tile([128, R], F32, name=f"tg_{t}_{j}_{rep}", tag=f"tg{j}")
                        nc.scalar.activation(tg, ps_g, TANH, bias=bg[:, 2 * KH + j:2 * KH + j + 1])
                        # c' = sigmoid(f)*c + sigmoid(i)*tanh(g)  (in-place scratch)
                        nc.vector.tensor_mul(sf, sf, c[j])
                        nc.vector.tensor_mul(si, si, tg)
                        cj = hp.tile([128, R], F32, name=f"c_{t}_{j}_{rep}", tag=f"c{j}")
                        nc.vector.tensor_add(cj, sf, si)
                        cn[j] = cj
                        tc_j = gp.tile([128, R], F32, name=f"tc_{t}_{j}_{rep}", tag=f"tc{j}")
                        nc.scalar.activation(tc_j, cj, TANH)
                        tanh_c[j] = tc_j
                    for j in range(KH):
                        # o chunks; h' = sigmoid(o) * tanh(c')
                        ps_o = emit_gate_group(3 * KH + j, xt, h, f"po_{t}_{j}_{rep}")
                        so = gp.tile([128, R], F32, name=f"so_{t}_{j}_{rep}", tag=f"so{j}")
                        nc.scalar.activation(so, ps_o, SIG, bias=bg[:, 3 * KH + j:3 * KH + j + 1])
                        hj = hp.tile([128, R], F32R, name=f"h_{t}_{j}_{rep}", tag=f"h{j}")
                        nc.vector.tensor_mul(hj, so, tanh_c[j])
                        hn[j] = hj
                    h, c = hn, cn

                # final step's logits
                ps_l = pl.tile([128, R], F32, name=f"plog_{C-1}_{rep}", tag="pl")
                for k in range(KH):
                    nc.tensor.matmul(ps_l, wo[k], h[k], start=(k == 0), stop=(k == KH - 1))
                flush_logits()
                prev_logits = (ps_l, C - 1)
                flush_logits()

    nc.compile()
    return nc


class Runner:
    """Compile once; execute the NEFF on 8 cores via PJRT repeatedly."""

    def __init__(self, nc):
        bass2jax.install_neuronx_cc_hook()
        self.nc = nc
        in_names, out_names, out_avals = [], [], []
        for alloc in nc.m.functions[0].allocations:
            if not isinstance(alloc, mybir.MemoryLocationSet):
                continue
            name = alloc.memorylocations[0].name
            if alloc.kind == "ExternalInput":
                in_names.append(name)
            elif alloc.kind == "ExternalOutput":
                out_names.append(name)
                out_avals.append(
                    jax.core.ShapedArray(tuple(alloc.tensor_shape), mybir.dt.np(alloc.dtype))
                )
        partition_name = nc.partition_id_tensor.name if nc.partition_id_tensor else None
        if partition_name is not None:
            in_names = [n for n in in_names if n != partition_name]
        self.in_names = in_names
        self.out_names = out_names
        self.out_avals = out_avals
        all_names = in_names + out_names + ([partition_name] if partition_name else [])
        n_params = len(in_names)
        n_outs = len(out_names)

        def _body(*args):
            operands = list(args)
            if partition_name is not None:
                operands.append(bass2jax.partition_id_tensor())
            outs = bass2jax._bass_exec_p.bind(
                *operands,
                out_avals=tuple(out_avals),
                in_names=tuple(all_names),
                out_names=tuple(out_names),
                lowering_input_output_aliases=(),
                sim_require_finite=True,
                sim_require_nnan=True,
                nc=nc,
            )
            return tuple(outs)

        devices = jax.devices()[:NCORES]
        mesh = Mesh(np.asarray(devices), ("core",))
        in_specs = (PartitionSpec("core"),) * (n_params + n_outs)
        out_specs = (PartitionSpec("core"),) * n_outs
        self._fn = jax.jit(
            shard_map(_body, mesh=mesh, in_specs=in_specs, out_specs=out_specs, check_rep=False)
        )
        self._zeros = [
            np.zeros((NCORES * a.shape[0], *a.shape[1:]), a.dtype) for a in out_avals
        ]

    def __call__(self, in_maps):
        concat_in = [
            np.concatenate([np.asarray(m[n]) for m in in_maps], axis=0)
            for n in self.in_names
        ]
        outs = self._fn(*concat_in, *self._zeros)
        jax.block_until_ready(outs)
        return [
            {
                n: np.asarray(outs[i]).reshape(NCORES, *self.out_avals[i].shape)[cidx]
                for i, n in enumerate(self.out_names)
            }
            for cidx in range(NCORES)
        ]


_RUNNER = None


def _get_runner():
    global _RUNNER
    if _RUNNER is None:
        _RUNNER = Runner(build_nc(repeat=1))
    return _RUNNER


def make_in_maps(quantized_repr, target_chars, Wp, bp, W_ih, W_hh, b_ih, b_hh, Wo, bo):
    qr = np.asarray(quantized_repr, np.float32).reshape(B * W, Q)
    chars = np.asarray(target_chars).reshape(B * W, C)
    valid = (chars >= 0) & (chars < A)
    chars = np.where(valid, chars, 0).astype(np.int64)

    Wp = np.asarray(Wp, np.float32)
    W_ih = np.asarray(W_ih, np.float32)
    W_hh = np.asarray(W_hh, np.float32)
    Wo = np.asarray(Wo, np.float32)
    b = (np.asarray(b_ih, np.float32) + np.asarray(b_hh, np.float32))

    shared = {
        "wpT": np.ascontiguousarray(Wp.T.reshape(KQ, 128, H)),
        "wihT": np.ascontiguousarray(W_ih.T),
        "whhT": np.ascontiguousarray(W_hh.T.reshape(KH, 128, 4 * H)),
        "woT": np.ascontiguousarray(Wo.T.reshape(KH, 128, A)),
        "bg": np.ascontiguousarray(b.reshape(NG, 128).T),
        "bp": np.ascontiguousarray(np.asarray(bp, np.float32).reshape(KH, 128).T),
        "bo": np.ascontiguousarray(np.asarray(bo, np.float32).reshape(A, 1)),
    }
    in_maps = []
    cols = np.arange(R)
    for cidx in range(NCORES):
        rows = slice(cidx * R, (cidx + 1) * R)
        qr_c = qr[rows]                      # [R, Q]
        ch_c = chars[rows]                   # [R, C]
        xT = np.zeros((C, A, R), np.float32)
        xT[0, 0, :] = 1.0                    # start token one-hot
        for t in range(1, C):
            xT[t, ch_c[:, t - 1], cols] = 1.0
        m = dict(shared)
        m["qrT"] = np.ascontiguousarray(qr_c.T.reshape(KQ, 128, R))
        m["xT"] = xT
        in_maps.append(m)
    return in_maps


def kernel(**inputs):
    runner = _get_runner()
    in_maps = make_in_maps(**inputs)
    results = runner(in_maps)
    # outT per core: [C, A, R] -> [R, C, A]; stack cores over rows
    parts = [res["outT"].transpose(2, 0, 1) for res in results]
    full = np.concatenate(parts, axis=0)      # [B*W, C, A]
    return np.ascontiguousarray(full.reshape(B, W, C, A).astype(np.float32))


# revision 8
# speedup vs baseline: 24.3984x; 24.3984x over previous
"""Trainium2 Bass kernel for nn_CharDecoder.

Teacher-forced character LSTM decoder:
  h0 = qr @ Wp.T + bp; c0 = 0
  per step: gates = x @ W_ih.T + h @ W_hh.T + b; LSTM cell; logits = h @ Wo.T + bo

Strategy:
 - Data-parallel over the flattened B*W = 4096 rows: 512 rows per core x 8 cores.
 - Everything on-device lives in transposed ("feature-major") layout
   [feature, row]: gatesT = W_ih @ xT + W_hh @ hT. The LSTM nonlinearities
   then produce hT directly in the lhs-contraction layout the next step's
   matmul needs -> no per-step transposes. The per-gate bias is a
   per-partition constant in this layout, folded into the ScalarE
   activation for free.
 - Matmuls run in float32r (full fp32 storage, reduced-precision PE mode):
   measured ~269 ns per [128x128]x[128x512] matmul vs ~1016 ns for fp32,
   with ~1e-4 relative error (vs ~2e-3 for bf16).
 - One-hot inputs are built on the host (same construction as the
   reference's jax.nn.one_hot input encoding); the matmuls consuming them
   run on device.
"""

import numpy as np
import jax
from jax.sharding import Mesh, PartitionSpec
from jax.experimental.shard_map import shard_map

import concourse.bacc as bacc
import concourse.mybir as mybir
import concourse.tile as tile
import concourse.bass2jax as bass2jax

B, W, Q, H, A, C = 32, 128, 256, 512, 128, 16
NCORES = 8
R = B * W // NCORES          # 512 rows per core
KH = H // 128                # 4 contraction chunks over H
KQ = Q // 128                # 2 contraction chunks over Q
NG = 4 * H // 128            # 16 gate-dim chunks (i,f,g,o x 4)

F32 = mybir.dt.float32
F32R = mybir.dt.float32r
SIG = mybir.ActivationFunctionType.Sigmoid
TANH = mybir.ActivationFunctionType.Tanh
IDENT = mybir.ActivationFunctionType.Identity


def build_nc(repeat=1, layered=True):
    nc = bacc.Bacc("TRN2", target_bir_lowering=False, debug=False, num_devices=NCORES)

    qrT_d = nc.dram_tensor("qrT", [KQ, 128, R], F32R, kind="ExternalInput").ap()
    xT_d = nc.dram_tensor("xT", [C, 128, R], F32R, kind="ExternalInput").ap()
    wpT_d = nc.dram_tensor("wpT", [KQ, 128, H], F32R, kind="ExternalInput").ap()
    wihT_d = nc.dram_tensor("wihT", [128, 4 * H], F32R, kind="ExternalInput").ap()
    whhT_d = nc.dram_tensor("whhT", [KH, 128, 4 * H], F32R, kind="ExternalInput").ap()
    woT_d = nc.dram_tensor("woT", [KH, 128, A], F32R, kind="ExternalInput").ap()
    bg_d = nc.dram_tensor("bg", [128, NG], F32, kind="ExternalInput").ap()
    bp_d = nc.dram_tensor("bp", [128, KH], F32, kind="ExternalInput").ap()
    bo_d = nc.dram_tensor("bo", [128, 1], F32, kind="ExternalInput").ap()
    out_d = nc.dram_tensor("outT", [C, 128, R], F32, kind="ExternalOutput").ap()

    with tile.TileContext(nc) as tc:
        with (
            tc.tile_pool(name="wpool", bufs=1) as wp,
            tc.tile_pool(name="hcpool", bufs=2) as hp,
            tc.tile_pool(name="gpool", bufs=1) as gp,
            tc.tile_pool(name="lpool", bufs=2) as lp,
            tc.tile_pool(name="pgates", bufs=6, space="PSUM") as pg,
            tc.tile_pool(name="plog", bufs=2, space="PSUM") as pl,
        ):
            # ---- persistent loads, ordered by when the compute needs them:
            # h0 needs wpt+qr; step 0 then needs wih+xt0+whh; wo/xt1.. later.
            wpt = []
            for q in range(KQ):
                t_ = wp.tile([128, H], F32R, name=f"wpt_{q}", tag=f"wpt{q}")
                nc.sync.dma_start(out=t_, in_=wpT_d[q])
                wpt.append(t_)
            qr = []
            for q in range(KQ):
                t_ = wp.tile([128, R], F32R, name=f"qr_{q}", tag=f"qr{q}")
                nc.sync.dma_start(out=t_, in_=qrT_d[q])
                qr.append(t_)
            bg = wp.tile([128, NG], F32, name="bg", tag="bg")
            nc.sync.dma_start(out=bg, in_=bg_d)
            bp = wp.tile([128, KH], F32, name="bp", tag="bp")
            nc.sync.dma_start(out=bp, in_=bp_d)
            bo = wp.tile([128, 1], F32, name="bo", tag="bo")
            nc.sync.dma_start(out=bo, in_=bo_d)
            wih = wp.tile([128, 4 * H], F32R, name="wih", tag="wih")
            nc.sync.dma_start(out=wih, in_=wihT_d)
            xts = [None] * C
            xts[0] = wp.tile([128, R], F32R, name="xt_0", tag="xt0")
            nc.sync.dma_start(out=xts[0], in_=xT_d[0])
            whh = []
            for k in range(KH):
                t_ = wp.tile([128, 4 * H], F32R, name=f"whh_{k}", tag=f"whh{k}")
                nc.sync.dma_start(out=t_, in_=whhT_d[k])
                whh.append(t_)
            wo = []
            for k in range(KH):
                t_ = wp.tile([128, A], F32R, name=f"wo_{k}", tag=f"wo{k}")
                nc.sync.dma_start(out=t_, in_=woT_d[k])
                wo.append(t_)
            for t in range(1, C):
                t_ = wp.tile([128, R], F32R, name=f"xt_{t}", tag=f"xt{t}")
                nc.sync.dma_start(out=t_, in_=xT_d[t])
                xts[t] = t_

            for rep in range(repeat):
                # ---- h0 = Wp @ qrT + bp ; c0 = 0 ----
                h = [None] * KH
                c = [None] * KH
                for j in range(KH):
                    ph = pg.tile([128, R], F32, name=f"ph0_{j}_{rep}", tag="gp")
                    for q in range(KQ):
                        nc.tensor.matmul(
                            ph, wpt[q][:, j * 128:(j + 1) * 128], qr[q],
                            start=(q == 0), stop=(q == KQ - 1),
                        )
                    hj = hp.tile([128, R], F32R, name=f"h0_{j}_{rep}", tag=f"h{j}")
                    nc.scalar.activation(hj, ph, IDENT, bias=bp[:, j:j + 1])
                    h[j] = hj
                    cj = hp.tile([128, R], F32, name=f"c0_{j}_{rep}", tag=f"c{j}")
                    nc.vector.memset(cj, 0.0)
                    c[j] = cj

                prev_logits = None  # (psum_tile, step) pending bias-add + store

                def emit_gate_group(m, xt, hin, name):
                    ps = pg.tile([128, R], F32, name=name, tag="gp")
                    nc.tensor.matmul(
                        ps, wih[:, m * 128:(m + 1) * 128], xt, start=True, stop=False
                    )
                    for k in range(KH):
                        nc.tensor.matmul(
                            ps, whh[k][:, m * 128:(m + 1) * 128], hin[k],
                            start=False, stop=(k == KH - 1),
                        )
                    return ps

                def flush_logits():
                    nonlocal prev_logits
                    if prev_logits is None:
                        return
                    ps_l, t_l = prev_logits
                    lsb = lp.tile([128, R], F32, name=f"lsb_{t_l}_{rep}", tag="lsb")
                    nc.scalar.activation(lsb, ps_l, IDENT, bias=bo[:, 0:1])
                    nc.sync.dma_start(out=out_d[t_l], in_=lsb)
                    prev_logits = None

                for t in range(C):
                    xt = xts[t]
                    hn = [None] * KH
                    cn = [None] * KH
                    sig_o = [None] * KH

                    def emit_logits_prev():
                        nonlocal prev_logits
                        ps_l = pl.tile([128, R], F32, name=f"plog_{t-1}_{rep}", tag="pl")
                        for k in range(KH):
                            nc.tensor.matmul(
                                ps_l, wo[k], h[k], start=(k == 0), stop=(k == KH - 1)
                            )
                        flush_logits()
                        prev_logits = (ps_l, t - 1)

                    if layered:
                        # Open the four o-gate groups plus i0/f0 together and
                        # emit their x/k0/k1/k2 layers before any k3: ~6.5us
                        # of PE work independent of the last h-slice of the
                        # previous step, hiding its ACT/DVE chain latency.
                        six = [3 * KH + 0, 3 * KH + 1, 3 * KH + 2, 3 * KH + 3,
                               0 * KH + 0, 1 * KH + 0]
                        names = [f"po_{t}_0_{rep}", f"po_{t}_1_{rep}", f"po_{t}_2_{rep}",
                                 f"po_{t}_3_{rep}", f"pi_{t}_0_{rep}", f"pf_{t}_0_{rep}"]
                        tiles6 = []
                        for m, nm in zip(six, names):
                            ps = pg.tile([128, R], F32, name=nm, tag="gp")
                            nc.tensor.matmul(
                                ps, wih[:, m * 128:(m + 1) * 128], xt,
                                start=True, stop=False,
                            )
                            tiles6.append(ps)
                        for k in range(KH - 1):
                            for m, ps in zip(six, tiles6):
                                nc.tensor.matmul(
                                    ps, whh[k][:, m * 128:(m + 1) * 128], h[k],
                                    start=False, stop=False,
                                )
                        if t > 0:
                            emit_logits_prev()
                        acts6 = []
                        for m, ps in zip(six, tiles6):
                            nc.tensor.matmul(
                                ps, whh[KH - 1][:, m * 128:(m + 1) * 128], h[KH - 1],
                                start=False, stop=True,
                            )
                        for idx, (m, ps) in enumerate(zip(six, tiles6)):
                            if idx < 4:
                                j = m - 3 * KH
                                so = gp.tile([128, R], F32, name=f"so_{t}_{j}_{rep}", tag=f"so{j}")
                                nc.scalar.activation(so, ps, SIG, bias=bg[:, m:m + 1])
                                sig_o[j] = so
                        si0 = gp.tile([128, R], F32, name=f"si_{t}_0_{rep}", tag="si0")
                        nc.scalar.activation(si0, tiles6[4], SIG, bias=bg[:, 0:1])
                        sf0 = gp.tile([128, R], F32, name=f"sf_{t}_0_{rep}", tag="sf0")
                        nc.scalar.activation(sf0, tiles6[5], SIG, bias=bg[:, KH:KH + 1])
                        pre = {0: (si0, sf0)}
                    else:
                        pre = {}
                        for j in range(KH):
                            ps_o = emit_gate_group(3 * KH + j, xt, h, f"po_{t}_{j}_{rep}")
                            if t > 0 and j == 0:
                                emit_logits_prev()
                            so = gp.tile([128, R], F32, name=f"so_{t}_{j}_{rep}", tag=f"so{j}")
                            nc.scalar.activation(so, ps_o, SIG, bias=bg[:, 3 * KH + j:3 * KH + j + 1])
                            sig_o[j] = so
                    for j in range(KH):
                        # i, f, g chunks for this H-slice j
                        if j in pre:
                            si, sf = pre[j]
                        else:
                            ps_i = emit_gate_group(0 * KH + j, xt, h, f"pi_{t}_{j}_{rep}")
                            si = gp.tile([128, R], F32, name=f"si_{t}_{j}_{rep}", tag=f"si{j}")
                            nc.scalar.activation(si, ps_i, SIG, bias=bg[:, 0 * KH + j:0 * KH + j + 1])
                            ps_f = emit_gate_group(1 * KH + j, xt, h, f"pf_{t}_{j}_{rep}")
                            sf = gp.tile([128, R], F32, name=f"sf_{t}_{j}_{rep}", tag=f"sf{j}")
                            nc.scalar.activation(sf, ps_f, SIG, bias=bg[:, 1 * KH + j:1 * KH + j + 1])
                        ps_g = emit_gate_group(2 * KH + j, xt, h, f"pg_{t}_{j}_{rep}")
                        tg = gp.tile([128, R], F32, name=f"tg_{t}_{j}_{rep}", tag=f"tg{j}")
                        nc.scalar.activation(tg, ps_g, TANH, bias=bg[:, 2 * KH + j:2 * KH + j + 1])
                        # c' = sigmoid(f)*c + sigmoid(i)*tanh(g)  (in-place scratch)
                        nc.vector.tensor_mul(sf, sf, c[j])
                        nc.vector.tensor_mul(si, si, tg)
                        cj = hp.tile([128, R], F32, name=f"c_{t}_{j}_{rep}", tag=f"c{j}")
                        nc.vector.tensor_add(cj, sf, si)
                        cn[j] = cj
                        tc_j = gp.tile([128, R], F32, name=f"tc_{t}_{j}_{rep}", tag=f"tc{j}")
                        nc.scalar.activation(tc_j, cj, TANH)
                        # h' = sigmoid(o) * tanh(c') — h-slice j completes as
                        # soon as its own c-chain does, so the next step's
                        # k=j accumulation never waits on later slices.
                        hj = hp.tile([128, R], F32R, name=f"h_{t}_{j}_{rep}", tag=f"h{j}")
                        nc.vector.tensor_mul(hj, sig_o[j], tc_j)
                        hn[j] = hj
                    h, c = hn, cn

                # final step's logits
                ps_l = pl.tile([128, R], F32, name=f"plog_{C-1}_{rep}", tag="pl")
                for k in range(KH):
                    nc.tensor.matmul(ps_l, wo[k], h[k], start=(k == 0), stop=(k == KH - 1))
                flush_logits()
                prev_logits = (ps_l, C - 1)
                flush_logits()

    nc.compile()
    return nc


class Runner:
    """Compile once; execute the NEFF on 8 cores via PJRT repeatedly."""

    def __init__(self, nc):
        bass2jax.install_neuronx_cc_hook()
        self.nc = nc
        in_names, out_names, out_avals = [], [], []
        for alloc in nc.m.functions[0].allocations:
            if not isinstance(alloc, mybir.MemoryLocationSet):
                continue
            name = alloc.memorylocations[0].name
            if alloc.kind == "ExternalInput":
                in_names.append(name)
            elif alloc.kind == "ExternalOutput":
                out_names.append(name)
                out_avals.append(
                    jax.core.ShapedArray(tuple(alloc.tensor_shape), mybir.dt.np(alloc.dtype))
                )
        partition_name = nc.partition_id_tensor.name if nc.partition_id_tensor else None
        if partition_name is not None:
            in_names = [n for n in in_names if n != partition_name]
        self.in_names = in_names
        self.out_names = out_names
        self.out_avals = out_avals
        all_names = in_names + out_names + ([partition_name] if partition_name else [])
        n_params = len(in_names)
        n_outs = len(out_names)

        def _body(*args):
            operands = list(args)
            if partition_name is not None:
                operands.append(bass2jax.partition_id_tensor())
            outs = bass2jax._bass_exec_p.bind(
                *operands,
                out_avals=tuple(out_avals),
                in_names=tuple(all_names),
                out_names=tuple(out_names),
                lowering_input_output_aliases=(),
                sim_require_finite=True,
                sim_require_nnan=True,
                nc=nc,
            )
            return tuple(outs)

        devices = jax.devices()[:NCORES]
        mesh = Mesh(np.asarray(devices), ("core",))
        in_specs = (PartitionSpec("core"),) * (n_params + n_outs)
        out_specs = (PartitionSpec("core"),) * n_outs
        self._fn = jax.jit(
            shard_map(_body, mesh=mesh, in_specs=in_specs, out_specs=out_specs, check_rep=False)
        )
        self._zeros = [
            np.zeros((NCORES * a.shape[0], *a.shape[1:]), a.dtype) for a in out_avals
        ]

    def __call__(self, in_maps):
        outs = self._fn(*self._concat(in_maps), *self._zeros)
        jax.block_until_ready(outs)
        return self._split(outs)

    def _concat(self, in_maps):
        return [
            np.concatenate([np.asarray(m[n]) for m in in_maps], axis=0)
            for n in self.in_names
        ]

    def _split(self, outs):
        return [
            {
                n: np.asarray(outs[i]).reshape(NCORES, *self.out_avals[i].shape)[cidx]
                for i, n in enumerate(self.out_names)
            }
            for cidx in range(NCORES)
        ]

    def bind(self, in_maps):
        """Pre-upload inputs to device for repeated timing runs."""
        self._bound = [jax.device_put(a) for a in self._concat(in_maps)] + [
            jax.device_put(z) for z in self._zeros
        ]

    def run_bound(self):
        outs = self._fn(*self._bound)
        jax.block_until_ready(outs)
        return outs


_RUNNER = None


def _get_runner():
    global _RUNNER
    if _RUNNER is None:
        _RUNNER = Runner(build_nc(repeat=1))
    return _RUNNER


def make_in_maps(quantized_repr, target_chars, Wp, bp, W_ih, W_hh, b_ih, b_hh, Wo, bo):
    qr = np.asarray(quantized_repr, np.float32).reshape(B * W, Q)
    chars = np.asarray(target_chars).reshape(B * W, C)
    valid = (chars >= 0) & (chars < A)
    chars = np.where(valid, chars, 0).astype(np.int64)

    Wp = np.asarray(Wp, np.float32)
    W_ih = np.asarray(W_ih, np.float32)
    W_hh = np.asarray(W_hh, np.float32)
    Wo = np.asarray(Wo, np.float32)
    b = (np.asarray(b_ih, np.float32) + np.asarray(b_hh, np.float32))

    shared = {
        "wpT": np.ascontiguousarray(Wp.T.reshape(KQ, 128, H)),
        "wihT": np.ascontiguousarray(W_ih.T),
        "whhT": np.ascontiguousarray(W_hh.T.reshape(KH, 128, 4 * H)),
        "woT": np.ascontiguousarray(Wo.T.reshape(KH, 128, A)),
        "bg": np.ascontiguousarray(b.reshape(NG, 128).T),
        "bp": np.ascontiguousarray(np.asarray(bp, np.float32).reshape(KH, 128).T),
        "bo": np.ascontiguousarray(np.asarray(bo, np.float32).reshape(A, 1)),
    }
    in_maps = []
    cols = np.arange(R)
    for cidx in range(NCORES):
        rows = slice(cidx * R, (cidx + 1) * R)
        qr_c = qr[rows]                      # [R, Q]
        ch_c = chars[rows]                   # [R, C]
        xT = np.zeros((C, A, R), np.float32)
        xT[0, 0, :] = 1.0                    # start token one-hot
        for t in range(1, C):
            xT[t, ch_c[:, t - 1], cols] = 1.0
        m = dict(shared)
        m["qrT"] = np.ascontiguousarray(qr_c.T.reshape(KQ, 128, R))
        m["xT"] = xT
        in_maps.append(m)
    return in_maps


def kernel(**inputs):
    global _RUNNER
    runner = _get_runner()
    in_maps = make_in_maps(**inputs)
    try:
        results = runner(in_maps)
    except Exception:
        # One retry with a freshly built executable: a transient device or
        # transport error (e.g. NRT_EXEC_UNIT_UNRECOVERABLE after an aborted
        # run) clears once a new program is loaded.
        _RUNNER = None
        runner = _get_runner()
        results = runner(in_maps)
    # outT per core: [C, A, R] -> [R, C, A]; stack cores over rows
    parts = [res["outT"].transpose(2, 0, 1) for res in results]
    full = np.concatenate(parts, axis=0)      # [B*W, C, A]
    return np.ascontiguousarray(full.reshape(B, W, C, A).astype(np.float32))
